# revision 6
# baseline (speedup 1.0000x reference)
"""Hyperbolic (Poincare ball, c=1) bilinear 2x upsample on 8 trn2 cores.

Math: geodesic midpoint on the ball reduces to mid = P*x + Q*y with P, Q
per-pixel scalars from the three channel dots (|x|^2, |y|^2, <x,y>).
Cell centers are vertical midpoints of the horizontal midpoints, so three
midpoint passes total.

Sharding: pure data parallel, one image (batch element) per NeuronCore.

Per-core layout: SBUF tiles [H=128 partitions, C=64, Wc] with W innermost.
Horizontal neighbours are free-dim shifts; vertical neighbours use a
partition-shifted SBUF->SBUF DMA copy (engines cannot read operands at
mismatched partition offsets).  Channel dots reduce over the middle axis
via a transposed AP view (axis=X).  P/Q broadcast along C with step-0 APs.

Toolchain note: this walrus build encodes at most ONE sync-wait per ISA
instruction ("Too many sync wait commands" otherwise); SplitWaitTC hoists
extra waits onto standalone EventSemaphore instructions.
"""

import numpy as np

B, C, H, W = 8, 64, 128, 128
N_CORES = 8
F32 = None  # set after mybir import

_runner = None


# ---------------------------------------------------------------------------
# Tile context with wait splitting
# ---------------------------------------------------------------------------


def _make_split_wait_tc():
    import concourse.mybir as mybir
    from concourse.tile import TileContext

    class SplitWaitTC(TileContext):
        MAXW = 1

        def _commit_instruction(self, inst, lazy_reg_writes=True):
            si = inst.sync_info
            if (
                si is not None
                and len(si.on_wait) > self.MAXW
                and inst.engine != mybir.EngineType.Unassigned
                and not isinstance(
                    inst,
                    (
                        mybir.InstNoOp,
                        mybir.InstEventSemaphore,
                        mybir.InstUnconditionalBranch,
                    ),
                )
            ):
                waits = list(si.on_wait)
                extra, keep = waits[: -self.MAXW], waits[-self.MAXW :]
                for w in extra:
                    ev = mybir.InstEventSemaphore(
                        name=self.nc.get_next_instruction_name(),
                        engine=inst.engine,
                        ins=[],
                        outs=[],
                        sync_info=mybir.SyncInfo(on_wait=[w], on_update=[]),
                    )
                    super()._commit_instruction(ev, lazy_reg_writes=False)
                inst.sync_info = mybir.SyncInfo(
                    on_wait=keep, on_update=list(si.on_update)
                )
            return super()._commit_instruction(inst, lazy_reg_writes)

        def _drain_and_barrier(self, tick_clock, wait_clock):
            from concourse.vector_clock import ScopedClock

            probe = mybir.InstEventSemaphore(
                name=self.nc.get_next_instruction_name(),
                engine=mybir.EngineType.SP,
                ins=[],
                outs=[],
            )
            wait_clock.add_sem_waits(
                probe, ScopedClock({None: tick_clock.global_clock})
            )
            waits = list(probe.sync_info.on_wait) if probe.sync_info else []
            for w in waits:
                ev = mybir.InstEventSemaphore(
                    name=self.nc.get_next_instruction_name(),
                    engine=mybir.EngineType.SP,
                    ins=[],
                    outs=[],
                    sync_info=mybir.SyncInfo(on_wait=[w], on_update=[]),
                )
                self._add_instruction(ev)
            self.nc.sync.drain()
            self.nc.all_engine_barrier()
            assert self.sems is not None
            popped = self.nc._tile_sem_poison_stack.pop()
            assert popped is self._sem_poison
            self.nc.clear_and_free_semaphores(list(self.sems.allocated().values()))
            self.nc.all_engine_barrier()

    return SplitWaitTC


# ---------------------------------------------------------------------------
# Kernel body
# ---------------------------------------------------------------------------


def _emit_pq(nc, pool, mybir, x2, y2, xy, P, Q, shape):
    """Emit P,Q = _PQ(x2, y2, xy); all APs [128, *shape] f32.

    g = 1-2xy; be = 1-x2; D1 = g + x2*y2; r1 = 1/D1
    a1 = (g+y2)*r1; b1 = be*r1
    w2 = a1*(a1*x2 - 2*b1*xy) + b1*(b1*y2); s = sqrt(max(1-w2, 1e-30))
    u = 1/(1+s); h = 1 + 2*u*(b1*xy - a1*x2); s2 = u*u*w2
    r2 = 1/(h + x2*s2); p = (h+s2)*r2; q = be*u*r2
    P = p - q*a1; Q = q*b1
    """
    dt = mybir.dt.float32
    al = mybir.AluOpType
    AF = mybir.ActivationFunctionType

    def T(tag):
        t = pool.tile([128, *shape], dt, tag=tag, name=tag)
        return t[:]

    tt = lambda o, a, b, op: nc.vector.tensor_tensor(out=o, in0=a, in1=b, op=op)
    ts = lambda o, a, s1, op0, s2, op1: nc.vector.tensor_scalar(
        out=o, in0=a, scalar1=s1, scalar2=s2, op0=op0, op1=op1
    )

    g = T("pq_g")
    ts(g, xy, -2.0, al.mult, 1.0, al.add)
    be = T("pq_be")
    ts(be, x2, -1.0, al.mult, 1.0, al.add)
    t0 = T("pq_t0")
    tt(t0, x2, y2, al.mult)
    tt(t0, g, t0, al.add)  # D1
    r1 = T("pq_r1")
    nc.vector.reciprocal(r1, t0)
    a1 = T("pq_a1")
    tt(a1, g, y2, al.add)
    tt(a1, a1, r1, al.mult)
    b1 = T("pq_b1")
    tt(b1, be, r1, al.mult)
    t2 = T("pq_t2")
    tt(t2, a1, x2, al.mult)  # a1*x2
    t5 = T("pq_t5")
    tt(t5, b1, xy, al.mult)  # b1*xy
    t6 = T("pq_t6")
    nc.vector.scalar_tensor_tensor(
        out=t6, in0=t5, scalar=-2.0, in1=t2, op0=al.mult, op1=al.add
    )  # t2 - 2*t5
    tt(t6, a1, t6, al.mult)
    t3 = T("pq_t3")
    tt(t3, b1, y2, al.mult)
    tt(t3, b1, t3, al.mult)
    w2 = T("pq_w2")
    tt(w2, t6, t3, al.add)
    sarg = T("pq_sarg")
    ts(sarg, w2, -1.0, al.mult, 1.0, al.add)
    nc.vector.tensor_scalar_max(sarg, sarg, 1e-30)
    s = T("pq_s")
    nc.scalar.activation(out=s, in_=sarg, func=AF.Sqrt)
    u = T("pq_u")
    nc.vector.tensor_scalar_add(u, s, 1.0)
    nc.vector.reciprocal(u, u)
    t11 = T("pq_t11")
    tt(t11, t5, t2, al.subtract)  # b1*xy - a1*x2
    tt(t11, u, t11, al.mult)
    hh = T("pq_h")
    ts(hh, t11, 2.0, al.mult, 1.0, al.add)
    s2 = T("pq_s2")
    tt(s2, u, w2, al.mult)
    tt(s2, u, s2, al.mult)
    den = T("pq_den")
    tt(den, x2, s2, al.mult)
    tt(den, hh, den, al.add)
    r2 = T("pq_r2")
    nc.vector.reciprocal(r2, den)
    p = T("pq_p")
    tt(p, hh, s2, al.add)
    tt(p, p, r2, al.mult)
    q = T("pq_q")
    tt(q, be, u, al.mult)
    tt(q, q, r2, al.mult)
    tt(Q, q, b1, al.mult)
    tt(P, q, a1, al.mult)
    tt(P, p, P, al.subtract)


def _build_bass():
    import concourse.bass as bass
    import concourse.mybir as mybir

    dt = mybir.dt.float32
    al = mybir.AluOpType
    AF = mybir.ActivationFunctionType
    AX = mybir.AxisListType
    SplitWaitTC = _make_split_wait_tc()

    nc = bass.Bass()
    x_in = nc.declare_dram_parameter("x", [C, H, W], dt, isOutput=False)
    out_ext = nc.declare_dram_parameter("out", [C, 2 * H, 2 * W], dt, isOutput=True)
    # [2, H, C, 2W]: [0] = even output rows, [1] = odd output rows
    out_v = out_ext[:].rearrange("c (h t) w -> t h c w", t=2)

    with SplitWaitTC(nc) as tc:
        with tc.tile_pool(name="p", bufs=1) as pool, tc.tile_pool(
            name="px", bufs=2
        ) as poolx:
            # chunks: (w0, n_xcols, n_mh, n_mv)
            for w0, nwx, nmh, nmv in (
                (0, 45, 44, 44),
                (44, 45, 44, 44),
                (88, 40, 39, 40),
            ):
                xt = poolx.tile([128, 64, 45], dt, tag="x")
                xs = poolx.tile([128, 64, 45], dt, tag="xs")
                nc.sync.dma_start(
                    out=xt[:, :, :nwx],
                    in_=x_in[:, :, w0 : w0 + nwx].transpose([1, 0, 2]),
                )
                nc.sync.dma_start(
                    out=xs[0:127, :, :nwx],
                    in_=x_in[:, 1:128, w0 : w0 + nwx].transpose([1, 0, 2]),
                )
                nc.sync.dma_start(
                    out=xs[127:128, :, :nwx],
                    in_=x_in[:, 127:128, w0 : w0 + nwx].transpose([1, 0, 2]),
                )

                # --- channel dots -------------------------------------------
                sq = pool.tile([128, 64, 45], dt, tag="prod")
                nc.scalar.activation(
                    out=sq[:, :, :nwx], in_=xt[:, :, :nwx], func=AF.Square
                )
                S = pool.tile([128, 45], dt, tag="S")
                nc.vector.tensor_reduce(
                    out=S[:, :nwx].unsqueeze(2),
                    in_=sq[:, :, :nwx].transpose([0, 2, 1]),
                    axis=AX.X,
                    op=al.add,
                )
                Sd = pool.tile([128, 45], dt, tag="Sd")
                nc.sync.dma_start(out=Sd[0:127, :nwx], in_=S[1:128, :nwx])
                nc.sync.dma_start(out=Sd[127:128, :nwx], in_=S[127:128, :nwx])

                ph = pool.tile([128, 64, 45], dt, tag="prod2")
                nc.vector.tensor_tensor(
                    out=ph[:, :, :nmh],
                    in0=xt[:, :, 0:nmh],
                    in1=xt[:, :, 1 : nmh + 1],
                    op=al.mult,
                )
                Hh = pool.tile([128, 45], dt, tag="Hh")
                nc.vector.tensor_reduce(
                    out=Hh[:, :nmh].unsqueeze(2),
                    in_=ph[:, :, :nmh].transpose([0, 2, 1]),
                    axis=AX.X,
                    op=al.add,
                )
                pv = pool.tile([128, 64, 45], dt, tag="prod")
                nc.vector.tensor_tensor(
                    out=pv[:, :, :nmv],
                    in0=xt[:, :, :nmv],
                    in1=xs[:, :, :nmv],
                    op=al.mult,
                )
                Vv = pool.tile([128, 45], dt, tag="Vv")
                nc.vector.tensor_reduce(
                    out=Vv[:, :nmv].unsqueeze(2),
                    in_=pv[:, :, :nmv].transpose([0, 2, 1]),
                    axis=AX.X,
                    op=al.add,
                )

                # --- stacked P/Q for horizontal (lane 0) & vertical (lane 1)
                NW = 44
                x2s = pool.tile([128, 2, NW], dt, tag="x2s")
                y2s = pool.tile([128, 2, NW], dt, tag="y2s")
                xys = pool.tile([128, 2, NW], dt, tag="xys")
                cp = nc.vector.tensor_copy
                cp(out=x2s[:, 0, :nmh], in_=S[:, :nmh])
                cp(out=y2s[:, 0, :nmh], in_=S[:, 1 : nmh + 1])
                cp(out=xys[:, 0, :nmh], in_=Hh[:, :nmh])
                cp(out=x2s[:, 1, :nmv], in_=S[:, :nmv])
                cp(out=y2s[:, 1, :nmv], in_=Sd[:, :nmv])
                cp(out=xys[:, 1, :nmv], in_=Vv[:, :nmv])
                for lane, nl in ((0, nmh), (1, nmv)):
                    if nl < NW:  # pad with a valid degenerate triple
                        dsrc = S[:, nl : nl + 1].broadcast_to([128, NW - nl])
                        cp(out=x2s[:, lane, nl:NW], in_=dsrc)
                        cp(out=y2s[:, lane, nl:NW], in_=dsrc)
                        cp(out=xys[:, lane, nl:NW], in_=dsrc)
                Ps = pool.tile([128, 2, NW], dt, tag="Ps")
                Qs = pool.tile([128, 2, NW], dt, tag="Qs")
                _emit_pq(nc, pool, mybir, x2s[:], y2s[:], xys[:], Ps[:], Qs[:], [2, NW])

                # --- mh = Ph*x_l + Qh*x_r -----------------------------------
                mh = pool.tile([128, 64, 45], dt, tag="mh")
                tta = pool.tile([128, 64, 45], dt, tag="cmb1")
                ttb = pool.tile([128, 64, 45], dt, tag="cmb2")
                Pb = Ps[:, 0:1, :nmh].broadcast_to([128, 64, nmh])
                Qb = Qs[:, 0:1, :nmh].broadcast_to([128, 64, nmh])
                nc.vector.tensor_tensor(
                    out=tta[:, :, :nmh], in0=xt[:, :, 0:nmh], in1=Pb, op=al.mult
                )
                nc.vector.tensor_tensor(
                    out=ttb[:, :, :nmh], in0=xt[:, :, 1 : nmh + 1], in1=Qb, op=al.mult
                )
                nc.vector.tensor_tensor(
                    out=mh[:, :, :nmh],
                    in0=tta[:, :, :nmh],
                    in1=ttb[:, :, :nmh],
                    op=al.add,
                )

                # --- EVEN rows tile: x at even slots, mh at odd slots -------
                even = pool.tile([128, 64, NW, 2], dt, tag="even")
                nc.scalar.activation(
                    out=even[:, :, :nmv, 0], in_=xt[:, :, :nmv], func=AF.Copy
                )
                nc.scalar.activation(
                    out=even[:, :, :nmh, 1], in_=mh[:, :, :nmh], func=AF.Copy
                )
                if nmh < nmv:  # out col 255 = dup of x col 127
                    nc.scalar.activation(
                        out=even[:, :, nmv - 1, 1],
                        in_=xt[:, :, nmv - 1],
                        func=AF.Copy,
                    )

                # --- mv = Pv*x + Qv*x_down ----------------------------------
                odd = pool.tile([128, 64, NW, 2], dt, tag="odd")
                tva = pool.tile([128, 64, 45], dt, tag="cmb1")
                tvb = pool.tile([128, 64, 45], dt, tag="cmb2")
                Pvb = Ps[:, 1:2, :nmv].broadcast_to([128, 64, nmv])
                Qvb = Qs[:, 1:2, :nmv].broadcast_to([128, 64, nmv])
                nc.vector.tensor_tensor(
                    out=tva[:, :, :nmv], in0=xt[:, :, :nmv], in1=Pvb, op=al.mult
                )
                nc.vector.tensor_tensor(
                    out=tvb[:, :, :nmv], in0=xs[:, :, :nmv], in1=Qvb, op=al.mult
                )
                nc.vector.tensor_tensor(
                    out=odd[0:127, :, :nmv, 0],
                    in0=tva[0:127, :, :nmv],
                    in1=tvb[0:127, :, :nmv],
                    op=al.add,
                )

                # --- ctr = Pc*mh + Qc*mh_down -------------------------------
                mhs = pool.tile([128, 64, 45], dt, tag="mhs")
                nc.sync.dma_start(out=mhs[0:127, :, :nmh], in_=mh[1:128, :, :nmh])
                nc.sync.dma_start(
                    out=mhs[127:128, :, :nmh], in_=mh[127:128, :, :nmh]
                )
                sqm = pool.tile([128, 64, 45], dt, tag="prod2")
                nc.scalar.activation(
                    out=sqm[:, :, :nmh], in_=mh[:, :, :nmh], func=AF.Square
                )
                Smh = pool.tile([128, 45], dt, tag="Smh")
                nc.vector.tensor_reduce(
                    out=Smh[:, :nmh].unsqueeze(2),
                    in_=sqm[:, :, :nmh].transpose([0, 2, 1]),
                    axis=AX.X,
                    op=al.add,
                )
                Smhd = pool.tile([128, 45], dt, tag="Smhd")
                nc.sync.dma_start(out=Smhd[0:127, :nmh], in_=Smh[1:128, :nmh])
                nc.sync.dma_start(out=Smhd[127:128, :nmh], in_=Smh[127:128, :nmh])
                pvm = pool.tile([128, 64, 45], dt, tag="prod")
                nc.vector.tensor_tensor(
                    out=pvm[:, :, :nmh],
                    in0=mh[:, :, :nmh],
                    in1=mhs[:, :, :nmh],
                    op=al.mult,
                )
                Vmh = pool.tile([128, 45], dt, tag="Vmh")
                nc.vector.tensor_reduce(
                    out=Vmh[:, :nmh].unsqueeze(2),
                    in_=pvm[:, :, :nmh].transpose([0, 2, 1]),
                    axis=AX.X,
                    op=al.add,
                )
                Pc = pool.tile([128, 45], dt, tag="Pc")
                Qc = pool.tile([128, 45], dt, tag="Qc")
                _emit_pq(
                    nc,
                    pool,
                    mybir,
                    Smh[:, :nmh],
                    Smhd[:, :nmh],
                    Vmh[:, :nmh],
                    Pc[:, :nmh],
                    Qc[:, :nmh],
                    [nmh],
                )
                tca = pool.tile([128, 64, 45], dt, tag="cmb1")
                tcb = pool.tile([128, 64, 45], dt, tag="cmb2")
                Pcb = Pc[:, :nmh].unsqueeze(1).broadcast_to([128, 64, nmh])
                Qcb = Qc[:, :nmh].unsqueeze(1).broadcast_to([128, 64, nmh])
                nc.vector.tensor_tensor(
                    out=tca[:, :, :nmh], in0=mh[:, :, :nmh], in1=Pcb, op=al.mult
                )
                nc.vector.tensor_tensor(
                    out=tcb[:, :, :nmh], in0=mhs[:, :, :nmh], in1=Qcb, op=al.mult
                )
                nc.vector.tensor_tensor(
                    out=odd[0:127, :, :nmh, 1],
                    in0=tca[0:127, :, :nmh],
                    in1=tcb[0:127, :, :nmh],
                    op=al.add,
                )
                if nmh < nmv:  # out col 255 (odd rows) = dup of mv col 127
                    nc.vector.tensor_copy(
                        out=odd[0:127, :, nmv - 1, 1], in_=odd[0:127, :, nmv - 1, 0]
                    )

                # --- stores --------------------------------------------------
                nc.sync.dma_start(
                    out=out_v[0, :, :, 2 * w0 : 2 * w0 + 2 * nmv],
                    in_=even[:, :, :nmv, :].rearrange("p c w t -> p c (w t)"),
                )
                nc.sync.dma_start(
                    out=out_v[1, 0:127, :, 2 * w0 : 2 * w0 + 2 * nmv],
                    in_=odd[0:127, :, :nmv, :].rearrange("p c w t -> p c (w t)"),
                )
                # out row 255 = dup of row 254 (= even tile, partition 127)
                nc.sync.dma_start(
                    out=out_v[1, 127:128, :, 2 * w0 : 2 * w0 + 2 * nmv],
                    in_=even[127:128, :, :nmv, :].rearrange("p c w t -> p c (w t)"),
                )
    return nc


# ---------------------------------------------------------------------------
# Runner: compile once, execute many
# ---------------------------------------------------------------------------


class _Runner:
    def __init__(self):
        import jax
        import jax.numpy as jnp
        from jax.sharding import Mesh, PartitionSpec
        from jax.experimental.shard_map import shard_map
        import concourse.mybir as mybir
        from concourse import bass2jax

        bass2jax.install_neuronx_cc_hook()
        nc = _build_bass()
        self.nc = nc

        in_names, out_names, out_avals, zero_outs = [], [], [], []
        partition_name = (
            nc.partition_id_tensor.name if nc.partition_id_tensor else None
        )
        for alloc in nc.m.functions[0].allocations:
            if not isinstance(alloc, mybir.MemoryLocationSet):
                continue
            name = alloc.memorylocations[0].name
            if alloc.kind == "ExternalInput":
                if name != partition_name:
                    in_names.append(name)
            elif alloc.kind == "ExternalOutput":
                shape = list(alloc.tensor_shape)
                np_dt = mybir.dt.np(alloc.dtype)
                out_names.append(name)
                out_avals.append(jax.core.ShapedArray(shape, np_dt))
                zero_outs.append(np.zeros(shape, np_dt))
        self.in_names = list(in_names)
        self.out_names = out_names
        self.n_params = len(in_names)
        n_outs = len(out_names)
        all_in_names = in_names + out_names
        if partition_name is not None:
            all_in_names.append(partition_name)

        def _body(*args):
            operands = list(args)
            if partition_name is not None:
                operands.append(bass2jax.partition_id_tensor())
            outs = bass2jax._bass_exec_p.bind(
                *operands,
                out_avals=tuple(out_avals),
                in_names=tuple(all_in_names),
                out_names=tuple(out_names),
                lowering_input_output_aliases=(),
                sim_require_finite=True,
                sim_require_nnan=True,
                nc=nc,
            )
            return tuple(outs)

        devices = jax.devices()[:N_CORES]
        mesh = Mesh(np.asarray(devices), ("core",))
        donate = tuple(range(self.n_params, self.n_params + n_outs))
        self.jitted = jax.jit(
            shard_map(
                _body,
                mesh=mesh,
                in_specs=(PartitionSpec("core"),) * (self.n_params + n_outs),
                out_specs=(PartitionSpec("core"),) * n_outs,
                check_rep=False,
            ),
            donate_argnums=donate,
            keep_unused=True,
        )
        self.zero_shapes = [
            (N_CORES * z.shape[0], *z.shape[1:]) for z in zero_outs
        ]
        self.zero_dtypes = [z.dtype for z in zero_outs]
        self._next_zero = None

    def time_device(self, x, iters=20):
        import time as _time

        x_cat = np.ascontiguousarray(x, np.float32).reshape(B * C, H, W)
        zeros = [np.zeros(s, d) for s, d in zip(self.zero_shapes, self.zero_dtypes)]
        outs = self.jitted(x_cat, *zeros)  # warm + device-resident chain head
        [o.block_until_ready() for o in outs]
        t0 = _time.perf_counter()
        for _ in range(iters):
            outs = self.jitted(x_cat, *outs)
        [o.block_until_ready() for o in outs]
        dt = (_time.perf_counter() - t0) / iters
        self._next_zero = None
        return dt * 1e9

    def __call__(self, x):
        # x: (B, C, H, W) -> concat per-core inputs along axis 0
        x_cat = np.ascontiguousarray(x, np.float32).reshape(B * C, H, W)
        if self._next_zero is None:
            zeros = [
                np.zeros(s, d) for s, d in zip(self.zero_shapes, self.zero_dtypes)
            ]
        else:
            zeros = self._next_zero
        outs = self.jitted(x_cat, *zeros)
        # reuse freshly produced device buffers as next call's donated outputs
        self._next_zero = list(outs)
        o = np.asarray(outs[0]).reshape(N_CORES, C, 2 * H, 2 * W)
        return o


def kernel(x: np.ndarray) -> np.ndarray:
    global _runner
    if _runner is None:
        _runner = _Runner()
    return _runner(x)


if __name__ == "__main__":
    rng = np.random.default_rng(0)
    n = rng.standard_normal((B, C, H, W)).astype(np.float32)
    nn_ = np.sqrt(np.maximum(np.sum(n * n, axis=1, keepdims=True), 1e-15))
    xv = (0.7 * n * np.tanh(nn_) / nn_).astype(np.float32)
    got = kernel(xv)
    print("kernel ran:", got.shape, got.dtype)
